# revision 14
# baseline (speedup 1.0000x reference)
"""CAM (channel attention) module kernel for Trainium2, 8-core data-parallel.

Reference computation (per sample b):
    q = conv2d(x, Wq, stride2, 2x2) -> [C, 4096]
    k = conv2d(x, Wk, stride2, 2x2) -> [C, 4096]
    v = conv2d(x, Wv, 1x1)          -> [C, 16384]
    E = q @ k^T                      [C, C]
    att = softmax(rowmax(E) - E)   (== softmin over rows)
    out = att @ v -> [C, H, W]

Kernel strategy (one sample per NeuronCore, B=8 over 8 cores):
  - The softmax is extremely peaked (energy entries span +-200), so
    energy errors are amplified: q/k need ~13+ mantissa bits, ruling
    out single-pass bf16/fp16 convs.
  - Precision scheme (2.13 streaming passes instead of split-bf16's 3):
      pass 1 (fp16):      (8192*Wh) @ xh        [Wh = fp16(W), xh = fp16(x)]
      pass 2 (fp8 DoubleRow, one stream for BOTH residual terms):
                          (8192*Wl) @ e4m3(x) + (16*Wh) @ (512*xl)
    Both passes accumulate into the SAME PSUM bank (everything scaled
    by 512 so no separate combine is needed); the existing PSUM->SBUF
    copy applies scale=1/512 for free. e4m3 residual operands give
    ~15-bit effective q/k precision (fp16 residuals are 2^-11; their
    e4m3 encodings err at 2^-4 of that) -> modeled rel err ~1.3e-3 vs
    the 2e-2 gate. DoubleRow streams the operand pair at bf16 byte rate
    (~1.13x a bf16 matmul), so the conv phase drops from 3 passes to
    ~2.13.
  - ALL weight/x splitting happens ON THE HOST: xh (bf16) + x8 (fp8
    pair planes, tap-major [4,2,512] per band) total the same 8 MiB of
    input DMA as before. No device-side split chain: the first conv
    matmul is gated only by a 0.25 MiB weight DMA + the first xh piece.
  - conv emits main-pass-major (bf16 q/k interleaved, then fp8-DR q/k)
    so a piece's x8 DMA can land in the shadow of its bf16 matmuls.
    A bf16 ident accumulation chain pre-warms the PE clock while the
    first input pieces stream in.
  - conv produces q in [c, n] layout, then PE-transposes to [n, c]
    chunks for the energy contraction. The transpose + energy matmuls
    of each piece are spread one-per-conv-matmul-pair into the NEXT
    piece's conv stream so their stationary loads hide under the conv
    streams; bands 0 and 7 run as half-band pieces to shrink the
    pipeline head and tail.
  - energy e = q k^T in native fp32 (exact), accumulated over 32 chunk
    matmuls in one PSUM bank.
  - softmin via one DVE row-min + one ScalarE exp (bias=rowmin,
    scale=-1) with fused accumulated row-sum, then att = w * (1/z) on
    DVE. Normalizing att up front keeps every out-phase PSUM->SBUF
    copy a PLAIN copy (a fused scale would halve DVE copy speed, and
    the copies pace the store phase).
  - out = att @ (Wv x + bv) == (att Wv) @ x (+ bias): computes
    M^T = Wv^T att^T on PE ([128,128]), casts to bf16 once, and runs
    out = Mh@xh against the resident xh tiles (1 bf16 pass, one
    stationary load; the dropped lo terms cost ~2.2e-3 rel err total
    vs the 2e-2 gate). The phase is store-DMA-paced (~2.9us/band):
    per-band copies spread 3:1 over DVE/ScalarE and store descriptors
    all issue from the sync queue (ONE dma-queue family: every extra
    family costs ~16 queue-drain semaphores in the end-of-kernel
    barrier, ~3us of epilogue); the first and last bands store in 2
    pieces (finer splits shrink the per-partition DMA burst size below
    4KB and cost ~10% wire efficiency).
"""

import numpy as np

B, C, H, W = 8, 128, 128, 128
HW = H * W           # 16384
N_CORES = 8
NB = 8               # number of H-bands (16 input rows each)
BAND = HW // NB      # 2048 x columns per band
HALF = BAND // 2     # 1024
QN = (H // 2) * (W // 2)  # 4096 conv output positions
QCHUNK = QN // NB    # 512 conv outputs per band

OUT_PASSES = 1       # bf16 passes in the output matmul (1..2)
N_PREWARM = 34       # PE ramp filler from engine-preamble end to first conv
N_SOFTWARM = 8       # PE filler across the softmax serial chain
WSCALE = 8192.0      # power-of-2 scale keeping fp8 residuals in range

_CACHE = {}


def _build_program(with_qk_bias: bool, with_v_bias: bool):
    import concourse.tile as tile
    from concourse import bacc, mybir
    from concourse.masks import make_identity

    f32 = mybir.dt.float32
    bf16 = mybir.dt.bfloat16
    f16 = mybir.dt.float16
    f8 = mybir.dt.float8e4
    Ident = mybir.ActivationFunctionType.Identity
    CopyF = mybir.ActivationFunctionType.Copy
    DR = mybir.MatmulPerfMode.DoubleRow
    nc = bacc.Bacc(
        "TRN2", target_bir_lowering=False, debug=False, num_devices=N_CORES)

    # all splits are host-side: xh = bf16(x); x8 = fp8 pair planes
    # (e4m3(x), e4m3(512*xl)) tap-major per band; weights likewise.
    xh_d = nc.declare_dram_parameter("xh", [C, HW], f16, isOutput=False)
    x8_d = nc.declare_dram_parameter("x8", [C, NB, 4, 2, QCHUNK], f8,
                                     isOutput=False)
    wqkh_d = nc.declare_dram_parameter("wqkh", [C, 8 * C], f16,
                                       isOutput=False)
    wqk8_d = nc.declare_dram_parameter("wqk8", [C, 16 * C], f8,
                                       isOutput=False)
    wv_d = nc.declare_dram_parameter("wv", [C, C], f32, isOutput=False)
    if with_qk_bias:
        bq_d = nc.declare_dram_parameter("bq", [C, 1], f32, isOutput=False)
        bk_d = nc.declare_dram_parameter("bk", [C, 1], f32, isOutput=False)
    if with_v_bias:
        bv_d = nc.declare_dram_parameter("bv", [C, 1], f32, isOutput=False)
    out_d = nc.declare_dram_parameter("out", [C, HW], f32, isOutput=True)

    with tile.TileContext(nc) as tc:
        with (
            tc.tile_pool(name="const", bufs=1) as const,
            tc.tile_pool(name="xrp", bufs=1) as xrp,
            tc.tile_pool(name="qkT", bufs=1) as qkT,
            tc.tile_pool(name="stage", bufs=3) as stage,
            tc.tile_pool(name="oout", bufs=6) as oout,
            tc.tile_pool(name="small", bufs=2) as small,
            tc.tile_pool(name="pacc", bufs=5, space="PSUM") as pacc,
            tc.tile_pool(name="ptp", bufs=2, space="PSUM") as ptp,
            tc.tile_pool(name="psm", bufs=1, space="PSUM") as psm,
        ):
            # PE clock pre-warm: a memset-zero bf16 tile is ready ~2.7us
            # before make_identity's iota chain, so the tensor engine can
            # start its HAM busy-window right after the engine preamble
            # (~7.2us) instead of waiting for the identity cast (~8.9us).
            mz = const.tile([128, 128], bf16, tag="mz")
            nc.gpsimd.memset(mz, 0.0)
            pwacc = pacc.tile([128, 128], f32, tag="acc", name="pw")
            for dw in range(N_PREWARM):
                nc.tensor.matmul(pwacc, lhsT=mz, rhs=mz,
                                 start=(dw == 0), stop=(dw == N_PREWARM - 1))

            ident = const.tile([128, 128], f32, tag="ident")
            make_identity(nc, ident)

            # Startup: per-core DMA bandwidth is ONE shared resource, so
            # serialize ALL input loads on the sync queue in consumption
            # order: main bf16 weights, band-0 first halves, fp8 weights,
            # band-0 second halves, then the remaining bands.
            wqkh_sb = const.tile([C, 8 * C], f16, tag="wqkh")
            nc.sync.dma_start(out=wqkh_sb, in_=wqkh_d[:, :])
            wqh_sb = wqkh_sb[:, 0:4 * C]
            wkh_sb = wqkh_sb[:, 4 * C:8 * C]
            if with_qk_bias:
                bq_sb = const.tile([C, 1], f32, tag="bq")
                nc.sync.dma_start(out=bq_sb, in_=bq_d[:, :])
                bk_sb = const.tile([C, 1], f32, tag="bk")
                nc.sync.dma_start(out=bk_sb, in_=bk_d[:, :])
            xh_sb = [xrp.tile([C, BAND], f16, tag=f"xh{j}", name=f"xh{j}")
                     for j in range(NB)]
            x8_sb = [xrp.tile([C, 4, 2, QCHUNK], f8, tag=f"x8{j}",
                              name=f"x8{j}")
                     for j in range(NB)]
            # band 0 lands in half-band pieces (bf16 then fp8, matching
            # the pass-major conv order); later bands as whole transfers
            nc.sync.dma_start(out=xh_sb[0][:, 0:HALF], in_=xh_d[:, 0:HALF])
            nc.sync.dma_start(out=xh_sb[0][:, HALF:BAND],
                              in_=xh_d[:, HALF:BAND])
            wqk8_sb = const.tile([C, 16 * C], f8, tag="wqk8")
            nc.sync.dma_start(out=wqk8_sb, in_=wqk8_d[:, :])
            wq8_v = wqk8_sb[:, 0:8 * C].rearrange(
                "p (ab two c) -> p ab two c", ab=4, two=2)
            wk8_v = wqk8_sb[:, 8 * C:16 * C].rearrange(
                "p (ab two c) -> p ab two c", ab=4, two=2)
            nc.sync.dma_start(out=x8_sb[0][:, :, :, 0:QCHUNK // 2],
                              in_=x8_d[:, 0, :, :, 0:QCHUNK // 2])
            nc.sync.dma_start(out=x8_sb[0][:, :, :, QCHUNK // 2:QCHUNK],
                              in_=x8_d[:, 0, :, :, QCHUNK // 2:QCHUNK])
            for j in range(1, NB):
                s = slice(j * BAND, (j + 1) * BAND)
                nc.sync.dma_start(out=xh_sb[j], in_=xh_d[:, s])
                nc.sync.dma_start(out=x8_sb[j], in_=x8_d[:, j])
            wv_sb = const.tile([C, C], f32, tag="wv")
            nc.sync.dma_start(out=wv_sb, in_=wv_d[:, :])
            if with_v_bias:
                bv_sb = const.tile([C, 1], f32, tag="bv")
                nc.sync.dma_start(out=bv_sb, in_=bv_d[:, :])

            qT = [qkT.tile([128, QCHUNK], f32, tag=f"qT{j}", name=f"qT{j}")
                  for j in range(NB)]
            kT = [qkT.tile([128, QCHUNK], f32, tag=f"kT{j}", name=f"kT{j}")
                  for j in range(NB)]

            # energy accumulator lives across the whole conv phase:
            # energy chunk matmuls are interleaved into the conv stream
            # so the PE never sits in a low-duty phase.
            E = psm.tile([128, 128], f32, tag="E")
            e_idx = [0]

            def conv_mains(j, lo, ncols, fills):
                """fp16 main-pass matmuls (4 taps, q/k interleaved) for x
                columns [lo, lo+ncols) of band j; opens the PSUM
                accumulation group (scaled by 8192). Two deferred
                transpose/energy ops from earlier pieces are emitted
                after each q/k matmul pair so their stationary loads
                hide under the conv streams."""
                i_cnt = ncols // 256
                nout = ncols // 4
                xh_v = xh_sb[j][:, lo:lo + ncols].rearrange(
                    "p (i a w b) -> p i a w b", i=i_cnt, a=2, w=64, b=2)
                acc_q = pacc.tile([128, 512], f32, tag="acc",
                                  name=f"aq{j}_{lo}")
                acc_k = pacc.tile([128, 512], f32, tag="acc",
                                  name=f"ak{j}_{lo}")
                for ab in range(4):
                    a, bb = ab // 2, ab % 2
                    rhs = xh_v[:, :, a, :, bb]
                    nc.tensor.matmul(
                        acc_q[:, 0:nout],
                        lhsT=wqh_sb[:, ab * C:(ab + 1) * C], rhs=rhs,
                        start=(ab == 0), stop=False)
                    nc.tensor.matmul(
                        acc_k[:, 0:nout],
                        lhsT=wkh_sb[:, ab * C:(ab + 1) * C], rhs=rhs,
                        start=(ab == 0), stop=False)
                    for _ in range(2):
                        if fills:
                            fills.pop(0)()
                return acc_q, acc_k

            def conv_dr(j, lo, ncols, acc_q, acc_k, fills):
                """fp8 DoubleRow residual matmuls closing the band piece's
                accumulation group: one matmul per tap streams BOTH
                residual terms (plane pairs)."""
                nout = ncols // 4
                oo = lo // 4
                for ab in range(4):
                    rhs8 = x8_sb[j][:, ab, :, oo:oo + nout]
                    nc.tensor.matmul(
                        acc_q[:, 0:nout],
                        lhsT=wq8_v[:, ab], rhs=rhs8,
                        start=False, stop=(ab == 3), perf_mode=DR)
                    nc.tensor.matmul(
                        acc_k[:, 0:nout],
                        lhsT=wk8_v[:, ab], rhs=rhs8,
                        start=False, stop=(ab == 3), perf_mode=DR)
                    for _ in range(2):
                        if fills:
                            fills.pop(0)()

            def tp_energy_thunks(j, qoff, nout, qc, kc):
                """Deferred transpose + energy ops for one finished conv
                piece, as single-instruction thunks."""
                th = []
                state = {}

                def mk_tp(src, t, key):
                    def f():
                        if key not in state:
                            state[key] = ptp.tile(
                                [128, 512], f32, tag="tp",
                                name=f"tp{j}_{qoff}_{key}")
                        nc.tensor.transpose(
                            state[key][:, t * 128:(t + 1) * 128],
                            src[:, t * 128:(t + 1) * 128], ident)
                    return f

                def mk_copy(T_out, key):
                    def f():
                        nc.scalar.activation(
                            out=T_out[:, qoff:qoff + nout],
                            in_=state[key][:, 0:nout],
                            func=CopyF, bias=0.0, scale=1.0)
                    return f

                def mk_e(t):
                    def f():
                        o = qoff + t * 128
                        nc.tensor.matmul(
                            E,
                            lhsT=qT[j][:, o:o + 128],
                            rhs=kT[j][:, o:o + 128],
                            start=(e_idx[0] == 0),
                            stop=(e_idx[0] == NB * 4 - 1))
                        e_idx[0] += 1
                    return f

                for key, (T_out, src) in enumerate(((qT[j], qc),
                                                    (kT[j], kc))):
                    for t in range(nout // 128):
                        th.append(mk_tp(src, t, key))
                    th.append(mk_copy(T_out, key))
                for t in range(nout // 128):
                    th.append(mk_e(t))
                return th

            fills = []

            def finish_piece(j, lo, ncols, acc_q, acc_k):
                nout = ncols // 4
                qc = stage.tile([128, 512], f32, tag="qchunk",
                                name=f"qc{j}_{lo}")
                kc = stage.tile([128, 512], f32, tag="kchunk",
                                name=f"kc{j}_{lo}")
                # the 1/8192 unscale rides the PSUM->SBUF copy for free
                if with_qk_bias:
                    nc.scalar.activation(out=qc[:, 0:nout],
                                         in_=acc_q[:, 0:nout], func=Ident,
                                         bias=bq_sb[:, 0:1],
                                         scale=1.0 / WSCALE)
                    nc.scalar.activation(out=kc[:, 0:nout],
                                         in_=acc_k[:, 0:nout], func=Ident,
                                         bias=bk_sb[:, 0:1],
                                         scale=1.0 / WSCALE)
                else:
                    nc.scalar.activation(out=qc[:, 0:nout],
                                         in_=acc_q[:, 0:nout], func=CopyF,
                                         bias=0.0, scale=1.0 / WSCALE)
                    nc.scalar.activation(out=kc[:, 0:nout],
                                         in_=acc_k[:, 0:nout], func=CopyF,
                                         bias=0.0, scale=1.0 / WSCALE)
                # transposes + energy one piece behind, spread into the
                # next piece's conv stream
                fills.extend(tp_energy_thunks(j, lo // 4, nout, qc, kc))

            # band 0 runs both half-pieces' fp16 mains back-to-back so the
            # fp8 weight/x tensors can land in their shadow while the DMA
            # queue is still ramping (175->400 GB/s over the first ~10us)
            a0 = conv_mains(0, 0, HALF, fills)
            b0 = conv_mains(0, HALF, HALF, fills)
            conv_dr(0, 0, HALF, *a0, fills)
            finish_piece(0, 0, HALF, *a0)
            conv_dr(0, HALF, HALF, *b0, fills)
            finish_piece(0, HALF, HALF, *b0)
            for j in range(1, NB - 1):
                accs = conv_mains(j, 0, BAND, fills)
                conv_dr(j, 0, BAND, *accs, fills)
                finish_piece(j, 0, BAND, *accs)
            for lo, ncols in ((0, HALF), (HALF, HALF // 2),
                              (HALF + HALF // 2, HALF // 2)):
                accs = conv_mains(NB - 1, lo, ncols, fills)
                conv_dr(NB - 1, lo, ncols, *accs, fills)
                finish_piece(NB - 1, lo, ncols, *accs)
            for f in fills:
                f()

            # keep the PE busy through the softmax serial chain so the
            # clock gate doesn't re-throttle before the output matmuls
            # (results unused; inputs are long since ready)
            for dw in range(N_SOFTWARM):
                scratch = pacc.tile([128, 256], f32, tag="acc",
                                    name=f"warm{dw}")
                nc.tensor.matmul(
                    scratch, lhsT=wqh_sb[:, 0:128],
                    rhs=xh_sb[0][:, 0:256],
                    start=True, stop=True)

            # softmin over rows: att = exp(min-E) / z. Normalizing att
            # up front keeps every out-phase PSUM->SBUF copy a PLAIN
            # copy (DVE runs 2x faster than with a fused scale), which
            # is what paces the store phase.
            mmin = small.tile([128, 1], f32, tag="mmin")
            nc.vector.tensor_reduce(
                out=mmin, in_=E, axis=mybir.AxisListType.X,
                op=mybir.AluOpType.min)
            w_sb = small.tile([128, 128], f32, tag="w")
            zsum = small.tile([128, 1], f32, tag="z")
            nc.scalar.activation(
                out=w_sb, in_=E, func=mybir.ActivationFunctionType.Exp,
                bias=mmin[:, 0:1], scale=-1.0, accum_out=zsum[:, 0:1])
            # 1/z folds into the out-phase copies as a per-partition
            # scale, so the reciprocal runs OFF the critical path (in
            # parallel with the transpose + M^T matmul below)
            rz = small.tile([128, 1], f32, tag="rz")
            nc.vector.reciprocal(rz, zsum)

            attT_p = psm.tile([128, 128], f32, tag="E")
            nc.tensor.transpose(attT_p, w_sb, ident)
            attT = small.tile([128, 128], f32, tag="attT")
            nc.vector.tensor_copy(attT, attT_p)

            # M^T[c2, c] = sum_d Wv[d, c2] attT[d, c], cast to bf16
            # once; the dropped lo terms cost ~2.2e-3 rel err.
            MT_p = psm.tile([128, 128], f32, tag="E")
            nc.tensor.matmul(MT_p, lhsT=wv_sb, rhs=attT,
                             start=True, stop=True)
            Mh = small.tile([128, 128], f16, tag="Mh")
            nc.vector.tensor_copy(Mh, MT_p)

            if with_v_bias:
                abv_p = psm.tile([128, 1], f32, tag="E")
                nc.tensor.matmul(abv_p, lhsT=attT, rhs=bv_sb[:, 0:1],
                                 start=True, stop=True)
                abv = small.tile([128, 1], f32, tag="abv")
                nc.vector.tensor_scalar_mul(abv, abv_p, rz[:, 0:1])

            # out[c, n] = sum_c2 M[c, c2] x[c2, n] (+ bias) via bf16; one
            # stationary load of Mh covers everything, PSUM accumulators
            # rotate through the pool. The phase must be store-DMA-paced
            # (~2.9us/band), so the per-band copies spread 3:1 over
            # DVE/ScalarE (plain copies, ~1.1us + 0.7us) and the store
            # descriptors issue from the otherwise-idle sync/gpsimd
            # queues. The first band is split 2x and the last 4x to
            # shrink lead-in/tail.
            for j in range(NB):
                o_band = oout.tile([128, BAND], f32, tag="oband")
                o_ps = [pacc.tile([128, 512], f32, tag="acc",
                                  name=f"ops{j}_{s}")
                        for s in range(4)]
                for s in range(4):
                    nc.tensor.matmul(
                        o_ps[s], lhsT=Mh,
                        rhs=xh_sb[j][:, s * 512:(s + 1) * 512],
                        start=True, stop=True)
                for s in range(4):
                    dst = o_band[:, s * 512:(s + 1) * 512]
                    if s % 2 == 0:
                        nc.scalar.activation(
                            out=dst, in_=o_ps[s], func=Ident,
                            bias=abv[:, 0:1] if with_v_bias else 0.0,
                            scale=rz[:, 0:1])
                    elif with_v_bias:
                        nc.vector.tensor_scalar(
                            out=dst, in0=o_ps[s], scalar1=rz[:, 0:1],
                            scalar2=abv[:, 0:1],
                            op0=mybir.AluOpType.mult,
                            op1=mybir.AluOpType.add)
                    else:
                        nc.vector.tensor_scalar_mul(dst, o_ps[s], rz[:, 0:1])
                n_pieces = 2 if j in (0, NB - 1) else 1
                psz = BAND // n_pieces
                for h in range(n_pieces):
                    off = j * BAND + h * psz
                    nc.sync.dma_start(
                        out=out_d[:, off:off + psz],
                        in_=o_band[:, h * psz:(h + 1) * psz])

    nc.compile()
    return nc


def kernel(x, Wq, bq, Wk, bk, Wv, bv):
    import ml_dtypes
    from concourse.bass_utils import run_bass_kernel_spmd

    f16 = np.float16
    f8 = ml_dtypes.float8_e4m3

    x = np.ascontiguousarray(np.asarray(x, dtype=np.float32))
    Wq = np.asarray(Wq, dtype=np.float32)
    Wk = np.asarray(Wk, dtype=np.float32)
    Wv = np.asarray(Wv, dtype=np.float32)
    bq = np.asarray(bq, dtype=np.float32)
    bk = np.asarray(bk, dtype=np.float32)
    bv = np.asarray(bv, dtype=np.float32)

    with_qk_bias = bool(np.any(bq) or np.any(bk))
    with_v_bias = bool(np.any(bv))

    key = (with_qk_bias, with_v_bias)
    if key not in _CACHE:
        _CACHE[key] = _build_program(with_qk_bias, with_v_bias)
    nc = _CACHE[key]

    # weight prep: wT[cin, ab*128 + c] = W[c, cin, a, b].
    # main pass: 512*bf16(W); fp8 pair planes: (512*Wl, bf16(W)) per tap.
    def prep_qk(Wc):
        WT = Wc.transpose(1, 2, 3, 0).reshape(C, 4, C)  # [cin, tap, cout]
        Wh = WT.astype(f16).astype(np.float32)
        Wl = WT - Wh
        w_main = (WSCALE * Wh).astype(f16).reshape(C, 4 * C)
        w8 = np.stack([(WSCALE * Wl).astype(f8), (16.0 * Wh).astype(f8)],
                      axis=2)  # [cin, tap, 2, cout]
        return w_main, w8.reshape(C, 8 * C)

    wqh, wq8 = prep_qk(Wq)
    wkh, wk8 = prep_qk(Wk)
    wqkh = np.ascontiguousarray(np.concatenate([wqh, wkh], axis=1))
    wqk8 = np.ascontiguousarray(np.concatenate([wq8, wk8], axis=1))
    wv = np.ascontiguousarray(Wv.reshape(C, C))

    # x prep: xh = bf16(x) in [C, HW]; x8 = fp8 pair planes in tap-major
    # band layout [C, NB, tap, 2, 512] with pairing
    #   plane0 = e4m3(x)        <->  8192*Wl
    #   plane1 = e4m3(512*xl)   <->  16*fp16(W)   (product scale 8192)
    xf = x.reshape(B, C, HW)
    xh_host = xf.astype(f16)
    xv = x.reshape(B, C, NB, 8, 2, 64, 2)       # j hh a w b
    p0 = xv.astype(f8)
    p1 = (512.0 * (xv - xv.astype(f16).astype(np.float32))).astype(f8)
    pair = np.stack([p0, p1], axis=-1)          # B C j hh a w b two
    pair = pair.transpose(0, 1, 2, 4, 6, 7, 3, 5)  # B C j a b two hh w
    x8_host = np.ascontiguousarray(pair).reshape(B, C, NB, 4, 2, QCHUNK)

    in_maps = []
    for b in range(B):
        m = {
            "xh": np.ascontiguousarray(xh_host[b]),
            "x8": x8_host[b],
            "wqkh": wqkh,
            "wqk8": wqk8,
            "wv": wv,
        }
        if with_qk_bias:
            m["bq"] = np.ascontiguousarray(bq.reshape(C, 1))
            m["bk"] = np.ascontiguousarray(bk.reshape(C, 1))
        if with_v_bias:
            m["bv"] = np.ascontiguousarray(bv.reshape(C, 1))
        in_maps.append(m)

    res = run_bass_kernel_spmd(nc, in_maps, list(range(N_CORES)))
    out = np.stack([res.results[i]["out"] for i in range(N_CORES)])
    return out.reshape(B, C, H, W).astype(np.float32)


# revision 16
# speedup vs baseline: 1.1740x; 1.1740x over previous
"""CAM (channel attention) module kernel for Trainium2, 8-core data-parallel.

Reference computation (per sample b):
    q = conv2d(x, Wq, stride2, 2x2) -> [C, 4096]
    k = conv2d(x, Wk, stride2, 2x2) -> [C, 4096]
    v = conv2d(x, Wv, 1x1)          -> [C, 16384]
    E = q @ k^T                      [C, C]
    att = softmax(rowmax(E) - E)   (== softmin over rows)
    out = att @ v -> [C, H, W]

Kernel strategy (one sample per NeuronCore, B=8 over 8 cores):
  - The softmax is extremely peaked (energy entries span +-200), so
    energy errors are amplified: q/k need ~13+ mantissa bits, ruling
    out single-pass bf16/fp16 convs.
  - Precision scheme (2.13 streaming passes instead of split-bf16's 3):
      pass 1 (fp16):      (8192*Wh) @ xh        [Wh = fp16(W), xh = fp16(x)]
      pass 2 (fp8 DoubleRow, one stream for BOTH residual terms):
                          (8192*Wl) @ e4m3(x) + (16*Wh) @ (512*xl)
    Both passes accumulate into the SAME PSUM bank (everything scaled
    by 512 so no separate combine is needed); the existing PSUM->SBUF
    copy applies scale=1/512 for free. e4m3 residual operands give
    ~15-bit effective q/k precision (fp16 residuals are 2^-11; their
    e4m3 encodings err at 2^-4 of that) -> modeled rel err ~1.3e-3 vs
    the 2e-2 gate. DoubleRow streams the operand pair at bf16 byte rate
    (~1.13x a bf16 matmul), so the conv phase drops from 3 passes to
    ~2.13.
  - ALL weight/x splitting happens ON THE HOST: xh (bf16) + x8 (fp8
    pair planes, tap-major [4,2,512] per band) total the same 8 MiB of
    input DMA as before. No device-side split chain: the first conv
    matmul is gated only by a 0.25 MiB weight DMA + the first xh piece.
  - conv emits main-pass-major (bf16 q/k interleaved, then fp8-DR q/k)
    so a piece's x8 DMA can land in the shadow of its bf16 matmuls.
    A bf16 ident accumulation chain pre-warms the PE clock while the
    first input pieces stream in.
  - conv produces q in [c, n] layout, then PE-transposes to [n, c]
    chunks for the energy contraction. The transpose + energy matmuls
    of each piece are spread one-per-conv-matmul-pair into the NEXT
    piece's conv stream so their stationary loads hide under the conv
    streams; bands 0 and 7 run as half-band pieces to shrink the
    pipeline head and tail.
  - energy e = q k^T in native fp32 (exact), accumulated over 32 chunk
    matmuls in one PSUM bank.
  - softmin via one DVE row-min + one ScalarE exp (bias=rowmin,
    scale=-1) with fused accumulated row-sum, then att = w * (1/z) on
    DVE. Normalizing att up front keeps every out-phase PSUM->SBUF
    copy a PLAIN copy (a fused scale would halve DVE copy speed, and
    the copies pace the store phase).
  - out = att @ (Wv x + bv) == (att Wv) @ x (+ bias): computes
    M^T = Wv^T att^T on PE ([128,128]), casts to bf16 once, and runs
    out = Mh@xh against the resident xh tiles (1 bf16 pass, one
    stationary load; the dropped lo terms cost ~2.2e-3 rel err total
    vs the 2e-2 gate). The phase is store-DMA-paced (~2.9us/band):
    per-band copies spread 3:1 over DVE/ScalarE and store descriptors
    all issue from the sync queue (ONE dma-queue family: every extra
    family costs ~16 queue-drain semaphores in the end-of-kernel
    barrier, ~3us of epilogue); the first and last bands store in 2
    pieces (finer splits shrink the per-partition DMA burst size below
    4KB and cost ~10% wire efficiency).
"""

import numpy as np

B, C, H, W = 8, 128, 128, 128
HW = H * W           # 16384
N_CORES = 8
NB = 8               # number of H-bands (16 input rows each)
BAND = HW // NB      # 2048 x columns per band
HALF = BAND // 2     # 1024
QN = (H // 2) * (W // 2)  # 4096 conv output positions
QCHUNK = QN // NB    # 512 conv outputs per band

OUT_PASSES = 1       # bf16 passes in the output matmul (1..2)
N_PREWARM = 34       # PE ramp filler from engine-preamble end to first conv
N_SOFTWARM = 8       # PE filler across the softmax serial chain
WSCALE = 8192.0      # power-of-2 scale keeping fp8 residuals in range

_CACHE = {}


def _build_program(with_qk_bias: bool, with_v_bias: bool):
    import concourse.tile as tile
    from concourse import bacc, mybir
    from concourse.masks import make_identity

    f32 = mybir.dt.float32
    bf16 = mybir.dt.bfloat16
    f16 = mybir.dt.float16
    f8 = mybir.dt.float8e4
    Ident = mybir.ActivationFunctionType.Identity
    CopyF = mybir.ActivationFunctionType.Copy
    DR = mybir.MatmulPerfMode.DoubleRow
    nc = bacc.Bacc(
        "TRN2", target_bir_lowering=False, debug=False, num_devices=N_CORES)

    # all splits are host-side: xh = bf16(x); x8 = fp8 pair planes
    # (e4m3(x), e4m3(512*xl)) tap-major per band; weights likewise.
    xh_d = nc.declare_dram_parameter("xh", [C, HW], f16, isOutput=False)
    x8_d = nc.declare_dram_parameter("x8", [C, NB, 4, 2, QCHUNK], f8,
                                     isOutput=False)
    wqkh_d = nc.declare_dram_parameter("wqkh", [C, 8 * C], f16,
                                       isOutput=False)
    wqk8_d = nc.declare_dram_parameter("wqk8", [C, 16 * C], f8,
                                       isOutput=False)
    wv_d = nc.declare_dram_parameter("wv", [C, C], f32, isOutput=False)
    if with_qk_bias:
        bq_d = nc.declare_dram_parameter("bq", [C, 1], f32, isOutput=False)
        bk_d = nc.declare_dram_parameter("bk", [C, 1], f32, isOutput=False)
    if with_v_bias:
        bv_d = nc.declare_dram_parameter("bv", [C, 1], f32, isOutput=False)
    # the output is stored on the wire as fp16 (the host upcasts to
    # fp32): out = Mh@xh is already fp16-precision-limited, so the
    # store rounding is free (modeled 1.32e-3 vs 1.31e-3) and the
    # wire-limited store phase halves (8MB -> 4MB, ~21us -> ~10.5us)
    out_d = nc.declare_dram_parameter("out", [C, HW], f16, isOutput=True)

    with tile.TileContext(nc) as tc:
        with (
            tc.tile_pool(name="const", bufs=1) as const,
            tc.tile_pool(name="xrp", bufs=1) as xrp,
            tc.tile_pool(name="qkT", bufs=1) as qkT,
            tc.tile_pool(name="stage", bufs=3) as stage,
            tc.tile_pool(name="oout", bufs=6) as oout,
            tc.tile_pool(name="small", bufs=2) as small,
            tc.tile_pool(name="pacc", bufs=5, space="PSUM") as pacc,
            tc.tile_pool(name="ptp", bufs=2, space="PSUM") as ptp,
            tc.tile_pool(name="psm", bufs=1, space="PSUM") as psm,
        ):
            # PE clock pre-warm: a memset-zero bf16 tile is ready ~2.7us
            # before make_identity's iota chain, so the tensor engine can
            # start its HAM busy-window right after the engine preamble
            # (~7.2us) instead of waiting for the identity cast (~8.9us).
            mz = const.tile([128, 128], bf16, tag="mz")
            nc.gpsimd.memset(mz, 0.0)
            pwacc = pacc.tile([128, 128], f32, tag="acc", name="pw")
            for dw in range(N_PREWARM):
                nc.tensor.matmul(pwacc, lhsT=mz, rhs=mz,
                                 start=(dw == 0), stop=(dw == N_PREWARM - 1))

            ident = const.tile([128, 128], f32, tag="ident")
            make_identity(nc, ident)

            # Startup: per-core DMA bandwidth is ONE shared resource, so
            # serialize ALL input loads on the sync queue in consumption
            # order: main bf16 weights, band-0 first halves, fp8 weights,
            # band-0 second halves, then the remaining bands.
            wqkh_sb = const.tile([C, 8 * C], f16, tag="wqkh")
            nc.sync.dma_start(out=wqkh_sb, in_=wqkh_d[:, :])
            wqh_sb = wqkh_sb[:, 0:4 * C]
            wkh_sb = wqkh_sb[:, 4 * C:8 * C]
            if with_qk_bias:
                bq_sb = const.tile([C, 1], f32, tag="bq")
                nc.sync.dma_start(out=bq_sb, in_=bq_d[:, :])
                bk_sb = const.tile([C, 1], f32, tag="bk")
                nc.sync.dma_start(out=bk_sb, in_=bk_d[:, :])
            xh_sb = [xrp.tile([C, BAND], f16, tag=f"xh{j}", name=f"xh{j}")
                     for j in range(NB)]
            x8_sb = [xrp.tile([C, 4, 2, QCHUNK], f8, tag=f"x8{j}",
                              name=f"x8{j}")
                     for j in range(NB)]
            # band 0 lands in half-band pieces (bf16 then fp8, matching
            # the pass-major conv order); later bands as whole transfers
            nc.sync.dma_start(out=xh_sb[0][:, 0:HALF], in_=xh_d[:, 0:HALF])
            nc.sync.dma_start(out=xh_sb[0][:, HALF:BAND],
                              in_=xh_d[:, HALF:BAND])
            wqk8_sb = const.tile([C, 16 * C], f8, tag="wqk8")
            nc.sync.dma_start(out=wqk8_sb, in_=wqk8_d[:, :])
            wq8_v = wqk8_sb[:, 0:8 * C].rearrange(
                "p (ab two c) -> p ab two c", ab=4, two=2)
            wk8_v = wqk8_sb[:, 8 * C:16 * C].rearrange(
                "p (ab two c) -> p ab two c", ab=4, two=2)
            nc.sync.dma_start(out=x8_sb[0][:, :, :, 0:QCHUNK // 2],
                              in_=x8_d[:, 0, :, :, 0:QCHUNK // 2])
            nc.sync.dma_start(out=x8_sb[0][:, :, :, QCHUNK // 2:QCHUNK],
                              in_=x8_d[:, 0, :, :, QCHUNK // 2:QCHUNK])
            for j in range(1, NB):
                s = slice(j * BAND, (j + 1) * BAND)
                nc.sync.dma_start(out=xh_sb[j], in_=xh_d[:, s])
                nc.sync.dma_start(out=x8_sb[j], in_=x8_d[:, j])
            wv_sb = const.tile([C, C], f32, tag="wv")
            nc.sync.dma_start(out=wv_sb, in_=wv_d[:, :])
            if with_v_bias:
                bv_sb = const.tile([C, 1], f32, tag="bv")
                nc.sync.dma_start(out=bv_sb, in_=bv_d[:, :])

            qT = [qkT.tile([128, QCHUNK], f32, tag=f"qT{j}", name=f"qT{j}")
                  for j in range(NB)]
            kT = [qkT.tile([128, QCHUNK], f32, tag=f"kT{j}", name=f"kT{j}")
                  for j in range(NB)]

            # energy accumulator lives across the whole conv phase:
            # energy chunk matmuls are interleaved into the conv stream
            # so the PE never sits in a low-duty phase.
            E = psm.tile([128, 128], f32, tag="E")
            e_idx = [0]

            def conv_mains(j, lo, ncols, fills):
                """fp16 main-pass matmuls (4 taps, q/k interleaved) for x
                columns [lo, lo+ncols) of band j; opens the PSUM
                accumulation group (scaled by 8192). Two deferred
                transpose/energy ops from earlier pieces are emitted
                after each q/k matmul pair so their stationary loads
                hide under the conv streams."""
                i_cnt = ncols // 256
                nout = ncols // 4
                xh_v = xh_sb[j][:, lo:lo + ncols].rearrange(
                    "p (i a w b) -> p i a w b", i=i_cnt, a=2, w=64, b=2)
                acc_q = pacc.tile([128, 512], f32, tag="acc",
                                  name=f"aq{j}_{lo}")
                acc_k = pacc.tile([128, 512], f32, tag="acc",
                                  name=f"ak{j}_{lo}")
                for ab in range(4):
                    a, bb = ab // 2, ab % 2
                    rhs = xh_v[:, :, a, :, bb]
                    nc.tensor.matmul(
                        acc_q[:, 0:nout],
                        lhsT=wqh_sb[:, ab * C:(ab + 1) * C], rhs=rhs,
                        start=(ab == 0), stop=False)
                    nc.tensor.matmul(
                        acc_k[:, 0:nout],
                        lhsT=wkh_sb[:, ab * C:(ab + 1) * C], rhs=rhs,
                        start=(ab == 0), stop=False)
                    for _ in range(2):
                        if fills:
                            fills.pop(0)()
                return acc_q, acc_k

            def conv_dr(j, lo, ncols, acc_q, acc_k, fills):
                """fp8 DoubleRow residual matmuls closing the band piece's
                accumulation group: one matmul per tap streams BOTH
                residual terms (plane pairs)."""
                nout = ncols // 4
                oo = lo // 4
                for ab in range(4):
                    rhs8 = x8_sb[j][:, ab, :, oo:oo + nout]
                    nc.tensor.matmul(
                        acc_q[:, 0:nout],
                        lhsT=wq8_v[:, ab], rhs=rhs8,
                        start=False, stop=(ab == 3), perf_mode=DR)
                    nc.tensor.matmul(
                        acc_k[:, 0:nout],
                        lhsT=wk8_v[:, ab], rhs=rhs8,
                        start=False, stop=(ab == 3), perf_mode=DR)
                    for _ in range(2):
                        if fills:
                            fills.pop(0)()

            def tp_energy_thunks(j, qoff, nout, qc, kc):
                """Deferred transpose + energy ops for one finished conv
                piece, as single-instruction thunks."""
                th = []
                state = {}

                def mk_tp(src, t, key):
                    def f():
                        if key not in state:
                            state[key] = ptp.tile(
                                [128, 512], f32, tag="tp",
                                name=f"tp{j}_{qoff}_{key}")
                        nc.tensor.transpose(
                            state[key][:, t * 128:(t + 1) * 128],
                            src[:, t * 128:(t + 1) * 128], ident)
                    return f

                def mk_copy(T_out, key):
                    def f():
                        nc.scalar.activation(
                            out=T_out[:, qoff:qoff + nout],
                            in_=state[key][:, 0:nout],
                            func=CopyF, bias=0.0, scale=1.0)
                    return f

                def mk_e(t):
                    def f():
                        o = qoff + t * 128
                        nc.tensor.matmul(
                            E,
                            lhsT=qT[j][:, o:o + 128],
                            rhs=kT[j][:, o:o + 128],
                            start=(e_idx[0] == 0),
                            stop=(e_idx[0] == NB * 4 - 1))
                        e_idx[0] += 1
                    return f

                for key, (T_out, src) in enumerate(((qT[j], qc),
                                                    (kT[j], kc))):
                    for t in range(nout // 128):
                        th.append(mk_tp(src, t, key))
                    th.append(mk_copy(T_out, key))
                for t in range(nout // 128):
                    th.append(mk_e(t))
                return th

            fills = []

            def finish_piece(j, lo, ncols, acc_q, acc_k):
                nout = ncols // 4
                qc = stage.tile([128, 512], f32, tag="qchunk",
                                name=f"qc{j}_{lo}")
                kc = stage.tile([128, 512], f32, tag="kchunk",
                                name=f"kc{j}_{lo}")
                # the 1/8192 unscale rides the PSUM->SBUF copy for free
                if with_qk_bias:
                    nc.scalar.activation(out=qc[:, 0:nout],
                                         in_=acc_q[:, 0:nout], func=Ident,
                                         bias=bq_sb[:, 0:1],
                                         scale=1.0 / WSCALE)
                    nc.scalar.activation(out=kc[:, 0:nout],
                                         in_=acc_k[:, 0:nout], func=Ident,
                                         bias=bk_sb[:, 0:1],
                                         scale=1.0 / WSCALE)
                else:
                    nc.scalar.activation(out=qc[:, 0:nout],
                                         in_=acc_q[:, 0:nout], func=CopyF,
                                         bias=0.0, scale=1.0 / WSCALE)
                    nc.scalar.activation(out=kc[:, 0:nout],
                                         in_=acc_k[:, 0:nout], func=CopyF,
                                         bias=0.0, scale=1.0 / WSCALE)
                # transposes + energy one piece behind, spread into the
                # next piece's conv stream
                fills.extend(tp_energy_thunks(j, lo // 4, nout, qc, kc))

            # band 0 runs both half-pieces' fp16 mains back-to-back so the
            # fp8 weight/x tensors can land in their shadow while the DMA
            # queue is still ramping (175->400 GB/s over the first ~10us)
            a0 = conv_mains(0, 0, HALF, fills)
            b0 = conv_mains(0, HALF, HALF, fills)
            conv_dr(0, 0, HALF, *a0, fills)
            finish_piece(0, 0, HALF, *a0)
            conv_dr(0, HALF, HALF, *b0, fills)
            finish_piece(0, HALF, HALF, *b0)
            for j in range(1, NB - 1):
                accs = conv_mains(j, 0, BAND, fills)
                conv_dr(j, 0, BAND, *accs, fills)
                finish_piece(j, 0, BAND, *accs)
            for lo in (0, HALF):
                accs = conv_mains(NB - 1, lo, HALF, fills)
                conv_dr(NB - 1, lo, HALF, *accs, fills)
                finish_piece(NB - 1, lo, HALF, *accs)
            for f in fills:
                f()

            # keep the PE busy through the softmax serial chain so the
            # clock gate doesn't re-throttle before the output matmuls
            # (results unused; inputs are long since ready)
            for dw in range(N_SOFTWARM):
                scratch = pacc.tile([128, 256], f32, tag="acc",
                                    name=f"warm{dw}")
                nc.tensor.matmul(
                    scratch, lhsT=wqh_sb[:, 0:128],
                    rhs=xh_sb[0][:, 0:256],
                    start=True, stop=True)

            # softmin over rows: att = exp(min-E) / z. Normalizing att
            # up front keeps every out-phase PSUM->SBUF copy a PLAIN
            # copy (DVE runs 2x faster than with a fused scale), which
            # is what paces the store phase.
            mmin = small.tile([128, 1], f32, tag="mmin")
            nc.vector.tensor_reduce(
                out=mmin, in_=E, axis=mybir.AxisListType.X,
                op=mybir.AluOpType.min)
            w_sb = small.tile([128, 128], f32, tag="w")
            zsum = small.tile([128, 1], f32, tag="z")
            nc.scalar.activation(
                out=w_sb, in_=E, func=mybir.ActivationFunctionType.Exp,
                bias=mmin[:, 0:1], scale=-1.0, accum_out=zsum[:, 0:1])
            # 1/z folds into the out-phase copies as a per-partition
            # scale, so the reciprocal runs OFF the critical path (in
            # parallel with the transpose + M^T matmul below)
            rz = small.tile([128, 1], f32, tag="rz")
            nc.vector.reciprocal(rz, zsum)

            attT_p = psm.tile([128, 128], f32, tag="E")
            nc.tensor.transpose(attT_p, w_sb, ident)
            attT = small.tile([128, 128], f32, tag="attT")
            nc.vector.tensor_copy(attT, attT_p)

            # M^T[c2, c] = sum_d Wv[d, c2] attT[d, c], cast to bf16
            # once; the dropped lo terms cost ~2.2e-3 rel err.
            MT_p = psm.tile([128, 128], f32, tag="E")
            nc.tensor.matmul(MT_p, lhsT=wv_sb, rhs=attT,
                             start=True, stop=True)
            Mh = small.tile([128, 128], f16, tag="Mh")
            nc.vector.tensor_copy(Mh, MT_p)

            if with_v_bias:
                abv_p = psm.tile([128, 1], f32, tag="E")
                nc.tensor.matmul(abv_p, lhsT=attT, rhs=bv_sb[:, 0:1],
                                 start=True, stop=True)
                abv = small.tile([128, 1], f32, tag="abv")
                nc.vector.tensor_scalar_mul(abv, abv_p, rz[:, 0:1])

            # out[c, n] = sum_c2 M[c, c2] x[c2, n] (+ bias) via bf16; one
            # stationary load of Mh covers everything, PSUM accumulators
            # rotate through the pool. The phase must be store-DMA-paced
            # (~2.9us/band), so the per-band copies spread 3:1 over
            # DVE/ScalarE (plain copies, ~1.1us + 0.7us) and the store
            # descriptors issue from the otherwise-idle sync/gpsimd
            # queues. The first band is split 2x and the last 4x to
            # shrink lead-in/tail.
            for j in range(NB):
                o_band = oout.tile([128, BAND], f16, tag="oband")
                o_ps = [pacc.tile([128, 512], f32, tag="acc",
                                  name=f"ops{j}_{s}")
                        for s in range(4)]
                for s in range(4):
                    nc.tensor.matmul(
                        o_ps[s], lhsT=Mh,
                        rhs=xh_sb[j][:, s * 512:(s + 1) * 512],
                        start=True, stop=True)
                for s in range(4):
                    dst = o_band[:, s * 512:(s + 1) * 512]
                    if s % 2 == 0:
                        nc.scalar.activation(
                            out=dst, in_=o_ps[s], func=Ident,
                            bias=abv[:, 0:1] if with_v_bias else 0.0,
                            scale=rz[:, 0:1])
                    elif with_v_bias:
                        nc.vector.tensor_scalar(
                            out=dst, in0=o_ps[s], scalar1=rz[:, 0:1],
                            scalar2=abv[:, 0:1],
                            op0=mybir.AluOpType.mult,
                            op1=mybir.AluOpType.add)
                    else:
                        nc.vector.tensor_scalar_mul(dst, o_ps[s], rz[:, 0:1])
                n_pieces = 2 if j in (0, NB - 1) else 1
                psz = BAND // n_pieces
                for h in range(n_pieces):
                    off = j * BAND + h * psz
                    nc.sync.dma_start(
                        out=out_d[:, off:off + psz],
                        in_=o_band[:, h * psz:(h + 1) * psz])

    nc.compile()
    return nc


def kernel(x, Wq, bq, Wk, bk, Wv, bv):
    import ml_dtypes
    from concourse.bass_utils import run_bass_kernel_spmd

    f16 = np.float16
    f8 = ml_dtypes.float8_e4m3

    x = np.ascontiguousarray(np.asarray(x, dtype=np.float32))
    Wq = np.asarray(Wq, dtype=np.float32)
    Wk = np.asarray(Wk, dtype=np.float32)
    Wv = np.asarray(Wv, dtype=np.float32)
    bq = np.asarray(bq, dtype=np.float32)
    bk = np.asarray(bk, dtype=np.float32)
    bv = np.asarray(bv, dtype=np.float32)

    with_qk_bias = bool(np.any(bq) or np.any(bk))
    with_v_bias = bool(np.any(bv))

    key = (with_qk_bias, with_v_bias)
    if key not in _CACHE:
        _CACHE[key] = _build_program(with_qk_bias, with_v_bias)
    nc = _CACHE[key]

    # weight prep: wT[cin, ab*128 + c] = W[c, cin, a, b].
    # main pass: 512*bf16(W); fp8 pair planes: (512*Wl, bf16(W)) per tap.
    def prep_qk(Wc):
        WT = Wc.transpose(1, 2, 3, 0).reshape(C, 4, C)  # [cin, tap, cout]
        Wh = WT.astype(f16).astype(np.float32)
        Wl = WT - Wh
        w_main = (WSCALE * Wh).astype(f16).reshape(C, 4 * C)
        w8 = np.stack([(WSCALE * Wl).astype(f8), (16.0 * Wh).astype(f8)],
                      axis=2)  # [cin, tap, 2, cout]
        return w_main, w8.reshape(C, 8 * C)

    wqh, wq8 = prep_qk(Wq)
    wkh, wk8 = prep_qk(Wk)
    wqkh = np.ascontiguousarray(np.concatenate([wqh, wkh], axis=1))
    wqk8 = np.ascontiguousarray(np.concatenate([wq8, wk8], axis=1))
    wv = np.ascontiguousarray(Wv.reshape(C, C))

    # x prep: xh = bf16(x) in [C, HW]; x8 = fp8 pair planes in tap-major
    # band layout [C, NB, tap, 2, 512] with pairing
    #   plane0 = e4m3(x)        <->  8192*Wl
    #   plane1 = e4m3(512*xl)   <->  16*fp16(W)   (product scale 8192)
    xf = x.reshape(B, C, HW)
    xh_host = xf.astype(f16)
    xv = x.reshape(B, C, NB, 8, 2, 64, 2)       # j hh a w b
    p0 = xv.astype(f8)
    p1 = (512.0 * (xv - xv.astype(f16).astype(np.float32))).astype(f8)
    pair = np.stack([p0, p1], axis=-1)          # B C j hh a w b two
    pair = pair.transpose(0, 1, 2, 4, 6, 7, 3, 5)  # B C j a b two hh w
    x8_host = np.ascontiguousarray(pair).reshape(B, C, NB, 4, 2, QCHUNK)

    in_maps = []
    for b in range(B):
        m = {
            "xh": np.ascontiguousarray(xh_host[b]),
            "x8": x8_host[b],
            "wqkh": wqkh,
            "wqk8": wqk8,
            "wv": wv,
        }
        if with_qk_bias:
            m["bq"] = np.ascontiguousarray(bq.reshape(C, 1))
            m["bk"] = np.ascontiguousarray(bk.reshape(C, 1))
        if with_v_bias:
            m["bv"] = np.ascontiguousarray(bv.reshape(C, 1))
        in_maps.append(m)

    res = run_bass_kernel_spmd(nc, in_maps, list(range(N_CORES)))
    out = np.stack([res.results[i]["out"] for i in range(N_CORES)])
    return out.reshape(B, C, H, W).astype(np.float32)
